# revision 11
# baseline (speedup 1.0000x reference)
"""ChebyKAN linear layer on 8 Trainium2 NeuronCores.

Math: y[b,j] = sum_{i,k} T_k(tanh(x[b,i])) * C[i,j,k],  k = 0..8.

  - Device computes the PRODUCT basis Q = [T1, T1^2, T1*T2, T2^2, T2*T3,
    T3^2, T3*T4, T4^2] (squares on ACT, products on DVE; T2/T3/T4 are
    transient). Since T_2m = 2*Q_2m - 1 and T_2m+1 = 2*Q_2m+1 - T1, the
    affine corrections fold into the host-side coefficients:
       A_1 = C_1 - C_3 - C_5 - C_7,  A_k = 2*C_k (k>=2),
       bias_j = sum_i (C_0 - C_2 - C_4 - C_6 - C_8)[i,j]
    (bias added during PSUM eviction). Conditioning stays ~1e-4 under the
    PE's f32r rounding, unlike the monomial basis.
  - The k>=1 contraction is a (2048 x 4096) @ (4096 x 512) matmul per
    core, run as 512 accumulating PE matmuls in float32r (full PE rate at
    N=512, ~1e-4 relative accuracy).

Sharding: data-parallel over Bv (16384 -> 8 x 2048), cheby_coeffs
replicated (host-relaid-out). Host pre-transposes x so the contraction
index i lands on SBUF partitions.
"""

import json as _json

import numpy as np

# ---------------------------------------------------------------------------
# Container workarounds (inlined so kernel.py is self-contained):
#  1. walrus here refuses instructions carrying >1 sem-wait; hoist excess
#     waits onto NoOps inserted before the offender (same engine queue).
#  2. TileContext tail drain accumulates one wait per logical processor;
#     pre-split them the same way.
# ---------------------------------------------------------------------------

import concourse.bass as bass
import concourse.tile as tile
from concourse import mybir
from concourse._compat import with_exitstack
from concourse.bass_utils import run_bass_kernel_spmd
from concourse.vector_clock import ScopedClock, VectorClock

_MAX_WAITS = 1


def _legalize_bir_json(raw: bytes) -> bytes:
    bir = _json.loads(raw)
    changed = False
    for fn in bir.get("functions", []):
        for blk in fn.get("blocks", []):
            out = []
            for inst in blk.get("instructions", []):
                si = inst.get("sync_info")
                waits = (si or {}).get("on_wait") or []
                if len(waits) > _MAX_WAITS:
                    changed = True
                    excess, keep = waits[:-_MAX_WAITS], waits[-_MAX_WAITS:]
                    for j, w in enumerate(excess):
                        out.append(
                            {
                                "debug": inst.get("debug", 0),
                                "engine": inst["engine"],
                                "ins": [],
                                "name": f"{inst['name']}--w{j}",
                                "opcode": "NoOp",
                                "outs": [],
                                "sync_info": {"on_update": [], "on_wait": [w]},
                                "text_hint": "wait_split",
                            }
                        )
                    si["on_wait"] = keep
                out.append(inst)
            blk["instructions"] = out
    return _json.dumps(bir).encode() if changed else raw


def _patched_drain_and_barrier(self, tick_clock, wait_clock):
    gc = tick_clock.global_clock
    n = len(gc)
    for proc in range(n):
        t = gc[proc]
        if t <= 0:
            continue
        vec = [0] * n
        vec[proc] = t
        nop = self.nc.sync.nop(nofuse=True, hint="tail_drain_split")
        wait_clock.add_sem_waits(nop.ins, ScopedClock({None: VectorClock(vec)}))
    self.nc.sync.drain()
    self.nc.all_engine_barrier()
    assert self.sems is not None
    popped = self.nc._tile_sem_poison_stack.pop()
    assert popped is self._sem_poison
    self.nc.clear_and_free_semaphores(list(self.sems.allocated().values()))
    self.nc.all_engine_barrier()


def _apply_patches():
    if getattr(bass.Bass, "_cheby_patched", False):
        return
    orig = bass.Bass.to_json_bytes

    def patched(self, *a, **kw):
        return _legalize_bir_json(orig(self, *a, **kw))

    bass.Bass.to_json_bytes = patched
    tile.TileContext._drain_and_barrier = _patched_drain_and_barrier
    bass.Bass._cheby_patched = True


_apply_patches()

# ---------------------------------------------------------------------------
# Problem constants (hardcoded per the harness contract)
# ---------------------------------------------------------------------------
NCORES = 8
BV, DIM, K = 16384, 512, 9
BC = BV // NCORES          # 2048 rows per core
SC = 512                   # b-superchunk width
NSC = BC // SC             # 4 superchunks per core
NIC = DIM // 128           # 4 i-chunks
NCH = NIC * (K - 1)        # 32 contraction chunks (k = 1..8)

F32 = mybir.dt.float32
F32R = mybir.dt.float32r
AFT = mybir.ActivationFunctionType
ALU = mybir.AluOpType


def _build_nc():
    nc = bass.Bass()
    xt_d = nc.dram_tensor("xt", (DIM, BC), F32, kind="ExternalInput")
    cm_d = nc.dram_tensor("cmat", (NCH, 128, DIM), F32R, kind="ExternalInput")
    bi_d = nc.dram_tensor("bias", (128, DIM), F32, kind="ExternalInput")
    y_d = nc.dram_tensor("y", (BC, DIM), F32, kind="ExternalOutput")

    @with_exitstack
    def kern(ctx, tc):
        nc = tc.nc
        cpool = ctx.enter_context(tc.tile_pool(name="cmat", bufs=1))
        bpool = ctx.enter_context(tc.tile_pool(name="bias", bufs=1))
        xpool = ctx.enter_context(tc.tile_pool(name="x", bufs=3))
        upool = ctx.enter_context(tc.tile_pool(name="u", bufs=2))
        tpool = ctx.enter_context(tc.tile_pool(name="basis", bufs=48))
        ppool = ctx.enter_context(tc.tile_pool(name="ps", bufs=4, space="PSUM"))
        ypool = ctx.enter_context(tc.tile_pool(name="y", bufs=4))

        # split the coefficient load so the first matmuls only wait on the
        # first 1MB slice (chunks are consumed in order c = ic*8 + (k-1));
        # first two slices ride the fast HWDGE queue, the rest go via
        # gpsimd so the x loads (vector queue) are never stuck behind them
        cm_tiles = []
        cm_r = cm_d.rearrange("c p j -> p c j")
        G = 4
        for g in range(NCH // G):
            cmt = cpool.tile([128, G, DIM], F32R, tag=f"cmat{g}", name=f"cm{g}")
            eng = nc.sync if g < 4 else nc.gpsimd
            eng.dma_start(cmt[:], cm_r[:, g * G : (g + 1) * G, :])
            cm_tiles.append(cmt)
        bi = bpool.tile([128, DIM], F32, tag="bias")
        nc.gpsimd.dma_start(bi[:], bi_d[:])
        negone = bpool.tile([128, 1], F32, tag="negone")
        nc.gpsimd.memset(negone[:], -1.0)

        for s in range(NSC):
            basis = []  # basis[ic][k-1] = Q_k tiles (128, SC), f32r
            for ic in range(NIC):
                xt = xpool.tile([128, SC], F32, tag="x")
                nc.scalar.dma_start(
                    xt[:], xt_d[ic * 128 : (ic + 1) * 128, s * SC : (s + 1) * SC]
                )
                Q = [
                    tpool.tile([128, SC], F32R, tag="basis", name=f"B{s}_{ic}_{k}")
                    for k in range(8)
                ]
                basis.append(Q)
                t2 = upool.tile([128, SC], F32, tag="t2")
                t3 = upool.tile([128, SC], F32, tag="t3")
                t4 = upool.tile([128, SC], F32, tag="t4")
                t3a = upool.tile([128, SC], F32, tag="t3a")
                # Q1 = T1 = tanh(x)
                nc.scalar.activation(Q[0][:], xt[:], AFT.Tanh)
                # Q2 = T1^2
                nc.scalar.activation(Q[1][:], Q[0][:], AFT.Square)
                # T2 = 2*Q2 - 1
                nc.scalar.activation(t2[:], Q[1][:], AFT.Identity, scale=2.0, bias=negone[:])
                # Q3 = T1*T2
                nc.vector.tensor_mul(Q[2][:], Q[0][:], t2[:])
                # T3 = 2*Q3 - T1
                nc.vector.tensor_add(t3a[:], Q[2][:], Q[2][:])
                nc.vector.tensor_sub(t3[:], t3a[:], Q[0][:])
                # Q4 = T2^2
                nc.scalar.activation(Q[3][:], t2[:], AFT.Square)
                # T4 = 2*Q4 - 1
                nc.scalar.activation(t4[:], Q[3][:], AFT.Identity, scale=2.0, bias=negone[:])
                # Q5 = T2*T3
                nc.vector.tensor_mul(Q[4][:], t2[:], t3[:])
                # Q6 = T3^2
                nc.scalar.activation(Q[5][:], t3[:], AFT.Square)
                # Q7 = T3*T4
                nc.vector.tensor_mul(Q[6][:], t3[:], t4[:])
                # Q8 = T4^2
                nc.scalar.activation(Q[7][:], t4[:], AFT.Square)

            for bc in range(SC // 128):
                ps = ppool.tile([128, DIM], F32, tag="ps")
                for c in range(NCH):
                    ic, km1 = divmod(c, 8)
                    lhsT = basis[ic][km1][:, bc * 128 : (bc + 1) * 128]
                    nc.tensor.matmul(
                        ps[:],
                        lhsT,
                        cm_tiles[c // 4][:, c % 4, :],
                        start=(c == 0),
                        stop=(c == NCH - 1),
                    )
                yt = ypool.tile([128, DIM], F32, tag="y")
                nc.vector.tensor_add(yt[:], ps[:], bi[:])
                b0 = s * SC + bc * 128
                nc.sync.dma_start(y_d[b0 : b0 + 128, :], yt[:])

    with tile.TileContext(nc) as tc:
        kern(tc)
    return nc


_NC_CACHE = None


def _get_nc():
    global _NC_CACHE
    if _NC_CACHE is None:
        _NC_CACHE = _build_nc()
    return _NC_CACHE


def _prep_inputs(x, cheby_coeffs):
    C = np.asarray(cheby_coeffs, dtype=np.float32)
    # product-basis coefficient transform (see module docstring)
    A = np.empty_like(C)
    A[:, :, 0] = 0.0
    A[:, :, 1] = C[:, :, 1] - C[:, :, 3] - C[:, :, 5] - C[:, :, 7]
    for k in range(2, K):
        A[:, :, k] = 2.0 * C[:, :, k]
    bias_j = (
        (C[:, :, 0] - C[:, :, 2] - C[:, :, 4] - C[:, :, 6] - C[:, :, 8])
        .sum(axis=0, dtype=np.float64)
        .astype(np.float32)
    )
    # contraction chunk c = ic*8 + (k-1) holds A[ic*128:(ic+1)*128, :, k]
    cmat = np.empty((NCH, 128, DIM), np.float32)
    for ic in range(NIC):
        for k in range(1, K):
            cmat[ic * 8 + (k - 1)] = A[ic * 128 : (ic + 1) * 128, :, k]
    bias = np.ascontiguousarray(np.broadcast_to(bias_j, (128, DIM)))
    xT = np.asarray(x, dtype=np.float32).T  # (DIM, BV) view
    in_maps = []
    for c in range(NCORES):
        in_maps.append(
            {
                "xt": np.ascontiguousarray(xT[:, c * BC : (c + 1) * BC]),
                "cmat": cmat,
                "bias": bias,
            }
        )
    return in_maps


def kernel(x, cheby_coeffs, _trace=False, _tmpdir=None):
    nc = _get_nc()
    in_maps = _prep_inputs(x, cheby_coeffs)
    res = run_bass_kernel_spmd(
        nc,
        in_maps,
        core_ids=list(range(NCORES)),
        trace=_trace,
        tmpdir=_tmpdir,
    )
    y = np.concatenate([r["y"] for r in res.results], axis=0)
    if _trace:
        kernel.last_result = res
    return y


# revision 12
# speedup vs baseline: 1.0480x; 1.0480x over previous
"""ChebyKAN linear layer on 8 Trainium2 NeuronCores.

Math: y[b,j] = sum_{i,k} T_k(tanh(x[b,i])) * C[i,j,k],  k = 0..8.

  - Device computes the PRODUCT basis Q = [T1, T1^2, T1*T2, T2^2, T2*T3,
    T3^2, T3*T4, T4^2] (squares on ACT, products on DVE; T2/T3/T4 are
    transient). Since T_2m = 2*Q_2m - 1 and T_2m+1 = 2*Q_2m+1 - T1, the
    affine corrections fold into the host-side coefficients:
       A_1 = C_1 - C_3 - C_5 - C_7,  A_k = 2*C_k (k>=2),
       bias_j = sum_i (C_0 - C_2 - C_4 - C_6 - C_8)[i,j]
    (bias added during PSUM eviction). Conditioning stays ~1e-4 under the
    PE's f32r rounding, unlike the monomial basis.
  - The k>=1 contraction is a (2048 x 4096) @ (4096 x 512) matmul per
    core, run as 512 accumulating PE matmuls in float32r (full PE rate at
    N=512, ~1e-4 relative accuracy).

Sharding: data-parallel over Bv (16384 -> 8 x 2048), cheby_coeffs
replicated (host-relaid-out). Host pre-transposes x so the contraction
index i lands on SBUF partitions.
"""

import json as _json

import numpy as np

# ---------------------------------------------------------------------------
# Container workarounds (inlined so kernel.py is self-contained):
#  1. walrus here refuses instructions carrying >1 sem-wait; hoist excess
#     waits onto NoOps inserted before the offender (same engine queue).
#  2. TileContext tail drain accumulates one wait per logical processor;
#     pre-split them the same way.
# ---------------------------------------------------------------------------

import concourse.bass as bass
import concourse.tile as tile
from concourse import mybir
from concourse._compat import with_exitstack
from concourse.bass_utils import run_bass_kernel_spmd
from concourse.vector_clock import ScopedClock, VectorClock

_MAX_WAITS = 1


def _legalize_bir_json(raw: bytes) -> bytes:
    bir = _json.loads(raw)
    changed = False
    for fn in bir.get("functions", []):
        for blk in fn.get("blocks", []):
            out = []
            for inst in blk.get("instructions", []):
                si = inst.get("sync_info")
                waits = (si or {}).get("on_wait") or []
                if len(waits) > _MAX_WAITS:
                    changed = True
                    excess, keep = waits[:-_MAX_WAITS], waits[-_MAX_WAITS:]
                    for j, w in enumerate(excess):
                        out.append(
                            {
                                "debug": inst.get("debug", 0),
                                "engine": inst["engine"],
                                "ins": [],
                                "name": f"{inst['name']}--w{j}",
                                "opcode": "NoOp",
                                "outs": [],
                                "sync_info": {"on_update": [], "on_wait": [w]},
                                "text_hint": "wait_split",
                            }
                        )
                    si["on_wait"] = keep
                out.append(inst)
            blk["instructions"] = out
    return _json.dumps(bir).encode() if changed else raw


def _patched_drain_and_barrier(self, tick_clock, wait_clock):
    gc = tick_clock.global_clock
    n = len(gc)
    for proc in range(n):
        t = gc[proc]
        if t <= 0:
            continue
        vec = [0] * n
        vec[proc] = t
        nop = self.nc.sync.nop(nofuse=True, hint="tail_drain_split")
        wait_clock.add_sem_waits(nop.ins, ScopedClock({None: VectorClock(vec)}))
    self.nc.sync.drain()
    self.nc.all_engine_barrier()
    assert self.sems is not None
    popped = self.nc._tile_sem_poison_stack.pop()
    assert popped is self._sem_poison
    self.nc.clear_and_free_semaphores(list(self.sems.allocated().values()))
    self.nc.all_engine_barrier()


def _apply_patches():
    if getattr(bass.Bass, "_cheby_patched", False):
        return
    orig = bass.Bass.to_json_bytes

    def patched(self, *a, **kw):
        return _legalize_bir_json(orig(self, *a, **kw))

    bass.Bass.to_json_bytes = patched
    tile.TileContext._drain_and_barrier = _patched_drain_and_barrier
    bass.Bass._cheby_patched = True


_apply_patches()

# ---------------------------------------------------------------------------
# Problem constants (hardcoded per the harness contract)
# ---------------------------------------------------------------------------
NCORES = 8
BV, DIM, K = 16384, 512, 9
BC = BV // NCORES          # 2048 rows per core
SC = 512                   # b-superchunk width
NSC = BC // SC             # 4 superchunks per core
NIC = DIM // 128           # 4 i-chunks
NCH = NIC * (K - 1)        # 32 contraction chunks (k = 1..8)

F32 = mybir.dt.float32
F32R = mybir.dt.float32r
AFT = mybir.ActivationFunctionType
ALU = mybir.AluOpType


def _build_nc():
    nc = bass.Bass()
    xt_d = nc.dram_tensor("xt", (DIM, BC), F32, kind="ExternalInput")
    cm_d = nc.dram_tensor("cmat", (NCH, 128, DIM), F32R, kind="ExternalInput")
    bi_d = nc.dram_tensor("bias", (128, DIM), F32, kind="ExternalInput")
    y_d = nc.dram_tensor("y", (BC, DIM), F32, kind="ExternalOutput")

    @with_exitstack
    def kern(ctx, tc):
        nc = tc.nc
        cpool = ctx.enter_context(tc.tile_pool(name="cmat", bufs=1))
        bpool = ctx.enter_context(tc.tile_pool(name="bias", bufs=1))
        xpool = ctx.enter_context(tc.tile_pool(name="x", bufs=3))
        upool = ctx.enter_context(tc.tile_pool(name="u", bufs=2))
        tpool = ctx.enter_context(tc.tile_pool(name="basis", bufs=48))
        ppool = ctx.enter_context(tc.tile_pool(name="ps", bufs=4, space="PSUM"))
        ypool = ctx.enter_context(tc.tile_pool(name="y", bufs=4))

        # split the coefficient load so the first matmuls only wait on the
        # first 1MB slice (chunks are consumed in order c = ic*8 + (k-1));
        # first two slices ride the fast HWDGE queue, the rest go via
        # gpsimd so the x loads (vector queue) are never stuck behind them
        cm_tiles = []
        cm_r = cm_d.rearrange("c p j -> p c j")
        G = 4
        for g in range(NCH // G):
            cmt = cpool.tile([128, G, DIM], F32R, tag=f"cmat{g}", name=f"cm{g}")
            eng = nc.sync if g < 2 else nc.gpsimd
            eng.dma_start(cmt[:], cm_r[:, g * G : (g + 1) * G, :])
            cm_tiles.append(cmt)
        bi = bpool.tile([128, DIM], F32, tag="bias")
        nc.gpsimd.dma_start(bi[:], bi_d[:])
        negone = bpool.tile([128, 1], F32, tag="negone")
        nc.gpsimd.memset(negone[:], -1.0)

        for s in range(NSC):
            basis = []  # basis[ic][k-1] = Q_k tiles (128, SC), f32r
            for ic in range(NIC):
                xt = xpool.tile([128, SC], F32, tag="x")
                nc.scalar.dma_start(
                    xt[:], xt_d[ic * 128 : (ic + 1) * 128, s * SC : (s + 1) * SC]
                )
                Q = [
                    tpool.tile([128, SC], F32R, tag="basis", name=f"B{s}_{ic}_{k}")
                    for k in range(8)
                ]
                basis.append(Q)
                t2 = upool.tile([128, SC], F32, tag="t2")
                t3 = upool.tile([128, SC], F32, tag="t3")
                t4 = upool.tile([128, SC], F32, tag="t4")
                t3a = upool.tile([128, SC], F32, tag="t3a")
                # Q1 = T1 = tanh(x)
                nc.scalar.activation(Q[0][:], xt[:], AFT.Tanh)
                # Q2 = T1^2
                nc.scalar.activation(Q[1][:], Q[0][:], AFT.Square)
                # T2 = 2*Q2 - 1
                nc.scalar.activation(t2[:], Q[1][:], AFT.Identity, scale=2.0, bias=negone[:])
                # Q3 = T1*T2
                nc.vector.tensor_mul(Q[2][:], Q[0][:], t2[:])
                # T3 = 2*Q3 - T1
                nc.vector.tensor_add(t3a[:], Q[2][:], Q[2][:])
                nc.vector.tensor_sub(t3[:], t3a[:], Q[0][:])
                # Q4 = T2^2
                nc.scalar.activation(Q[3][:], t2[:], AFT.Square)
                # T4 = 2*Q4 - 1
                nc.scalar.activation(t4[:], Q[3][:], AFT.Identity, scale=2.0, bias=negone[:])
                # Q5 = T2*T3
                nc.vector.tensor_mul(Q[4][:], t2[:], t3[:])
                # Q6 = T3^2
                nc.scalar.activation(Q[5][:], t3[:], AFT.Square)
                # Q7 = T3*T4
                nc.vector.tensor_mul(Q[6][:], t3[:], t4[:])
                # Q8 = T4^2
                nc.scalar.activation(Q[7][:], t4[:], AFT.Square)

            for bc in range(SC // 128):
                ps = ppool.tile([128, DIM], F32, tag="ps")
                for c in range(NCH):
                    ic, km1 = divmod(c, 8)
                    lhsT = basis[ic][km1][:, bc * 128 : (bc + 1) * 128]
                    nc.tensor.matmul(
                        ps[:],
                        lhsT,
                        cm_tiles[c // 4][:, c % 4, :],
                        start=(c == 0),
                        stop=(c == NCH - 1),
                    )
                yt = ypool.tile([128, DIM], F32, tag="y")
                nc.vector.tensor_add(yt[:], ps[:], bi[:])
                b0 = s * SC + bc * 128
                nc.sync.dma_start(y_d[b0 : b0 + 128, :], yt[:])

    with tile.TileContext(nc) as tc:
        kern(tc)
    return nc


_NC_CACHE = None


def _get_nc():
    global _NC_CACHE
    if _NC_CACHE is None:
        _NC_CACHE = _build_nc()
    return _NC_CACHE


def _prep_inputs(x, cheby_coeffs):
    C = np.asarray(cheby_coeffs, dtype=np.float32)
    # product-basis coefficient transform (see module docstring)
    A = np.empty_like(C)
    A[:, :, 0] = 0.0
    A[:, :, 1] = C[:, :, 1] - C[:, :, 3] - C[:, :, 5] - C[:, :, 7]
    for k in range(2, K):
        A[:, :, k] = 2.0 * C[:, :, k]
    bias_j = (
        (C[:, :, 0] - C[:, :, 2] - C[:, :, 4] - C[:, :, 6] - C[:, :, 8])
        .sum(axis=0, dtype=np.float64)
        .astype(np.float32)
    )
    # contraction chunk c = ic*8 + (k-1) holds A[ic*128:(ic+1)*128, :, k]
    cmat = np.empty((NCH, 128, DIM), np.float32)
    for ic in range(NIC):
        for k in range(1, K):
            cmat[ic * 8 + (k - 1)] = A[ic * 128 : (ic + 1) * 128, :, k]
    bias = np.ascontiguousarray(np.broadcast_to(bias_j, (128, DIM)))
    xT = np.asarray(x, dtype=np.float32).T  # (DIM, BV) view
    in_maps = []
    for c in range(NCORES):
        in_maps.append(
            {
                "xt": np.ascontiguousarray(xT[:, c * BC : (c + 1) * BC]),
                "cmat": cmat,
                "bias": bias,
            }
        )
    return in_maps


def kernel(x, cheby_coeffs, _trace=False, _tmpdir=None):
    nc = _get_nc()
    in_maps = _prep_inputs(x, cheby_coeffs)
    res = run_bass_kernel_spmd(
        nc,
        in_maps,
        core_ids=list(range(NCORES)),
        trace=_trace,
        tmpdir=_tmpdir,
    )
    y = np.concatenate([r["y"] for r in res.results], axis=0)
    if _trace:
        kernel.last_result = res
    return y
